# revision 1
# baseline (speedup 1.0000x reference)
"""Trainium2 Bass kernel for ChatGLM attention (S=2048, B=1, H=4096, NH=32, HD=128).

Sharding: tensor-parallel over heads across 8 NeuronCores (4 heads/core).
Each core computes its heads' QKV projection, RoPE, attention, and a
partial dense projection (contraction over its 512 hidden dims); the host
sums the 8 partials and adds the dense bias.

All matmuls run in bf16 (fp32 PSUM accumulation); RoPE tables and
elementwise arithmetic are fp32. Verified ~5e-3 scale-relative absmax
error vs the fp32 reference.
"""

import math
from contextlib import ExitStack

import ml_dtypes
import numpy as np

import concourse.bacc as bacc
import concourse.mybir as mybir
import concourse.tile as tile
from concourse.bass_utils import run_bass_kernel_spmd

S, B, H, NH, HD = 2048, 1, 4096, 32, 128
ROT = HD // 2
NCORES = 8
HPC = NH // NCORES          # heads per core = 4
QK_TILES = 2 * HPC          # q/k col tiles per core = 8
HT = H // 128               # hidden tiles = 32
ST = S // 128               # seq tiles = 16
SC = S // 512               # 512-wide seq chunks = 4

F32 = mybir.dt.float32
BF16 = mybir.dt.bfloat16
BF = ml_dtypes.bfloat16

_PROG_CACHE = {}


def _build_program(coeff: float, mode: str, debug: bool = False, phases: int = 3):
    """mode: 'causal' | 'none' | 'general'"""
    nc = bacc.Bacc("TRN2", target_bir_lowering=False, debug=False)

    # all weight-ish inputs are laid out partition-major by the host so each
    # logical tile load is one DMA with >=1KB contiguous bursts per partition
    hT = nc.dram_tensor("hT", [128, HT, S], BF16, kind="ExternalInput")
    w_qk = nc.dram_tensor("w_qk", [QK_TILES, 128, HT, 128], BF16, kind="ExternalInput")
    w_v = nc.dram_tensor("w_v", [128, HT, HPC * 128], BF16, kind="ExternalInput")
    b_qk = nc.dram_tensor("b_qk", [QK_TILES, 128], F32, kind="ExternalInput")
    b_v = nc.dram_tensor("b_v", [128, HPC * 128], F32, kind="ExternalInput")
    cs_q = nc.dram_tensor("cs_q", [128, 2, S], F32, kind="ExternalInput")   # cos, sin_eff
    cs_k = nc.dram_tensor("cs_k", [128, 2, S], F32, kind="ExternalInput")
    triu = nc.dram_tensor("triu", [128, 128], BF16, kind="ExternalInput")
    wdT = nc.dram_tensor("wdT", [HPC, 128, H], BF16, kind="ExternalInput")
    if mode == "general":
        m01 = nc.dram_tensor("m01", [ST, 128, S], BF16, kind="ExternalInput")
    F16 = mybir.dt.float16
    out_p = nc.dram_tensor("out_p", [S, H], F16, kind="ExternalOutput")
    if debug:
        dbg_q = nc.dram_tensor("dbg_q", [128, HPC, S], BF16, kind="ExternalOutput")
        dbg_k = nc.dram_tensor("dbg_k", [128, HPC, S], BF16, kind="ExternalOutput")
        dbg_v = nc.dram_tensor("dbg_v", [128, ST, HPC * 128], BF16, kind="ExternalOutput")
        dbg_p = nc.dram_tensor("dbg_p", [128, ST, S], BF16, kind="ExternalOutput")
        dbg_l = nc.dram_tensor("dbg_l", [1, S], F32, kind="ExternalOutput")
        dbg_c = nc.dram_tensor("dbg_c", [128, HPC, S], BF16, kind="ExternalOutput")

    def s0_of(t):
        return 128 * t if mode == "causal" else 0

    with tile.TileContext(nc) as tc, ExitStack() as ctx:
        const = ctx.enter_context(tc.tile_pool(name="const", bufs=1))
        psum = ctx.enter_context(tc.tile_pool(name="psum", bufs=4, space="PSUM"))
        scratch = ctx.enter_context(tc.tile_pool(name="scratch", bufs=4, space="DRAM"))

        b_qk_sb = const.tile([128, QK_TILES], F32)
        nc.sync.dma_start(b_qk_sb[:], b_qk[:].rearrange("t p -> p t"))
        b_v_sb = const.tile([128, HPC * 128], F32)
        nc.sync.dma_start(b_v_sb[:], b_v[:])
        triu_sb = const.tile([128, 128], BF16)
        nc.sync.dma_start(triu_sb[:], triu[:])
        ones_sb = const.tile([128, 1], BF16)
        nc.vector.memset(ones_sb[:], 1.0)
        qT_sb = const.tile([128, HPC, S], BF16)
        kT_sb = const.tile([128, HPC, S], BF16)
        v_sb = const.tile([128, ST, HPC * 128], BF16)

        # ---------------- Phase 1: QKV projection + RoPE ----------------
        with ExitStack() as qctx:
            wv_pool = qctx.enter_context(tc.tile_pool(name="wv", bufs=1))
            hid_pool = qctx.enter_context(tc.tile_pool(name="hid", bufs=2))
            cs_pool = qctx.enter_context(tc.tile_pool(name="cs", bufs=1))
            wqk_pool = qctx.enter_context(tc.tile_pool(name="wqk", bufs=2))
            rt_pool = qctx.enter_context(tc.tile_pool(name="rt", bufs=2))

            wv_sb = wv_pool.tile([128, HT, HPC * 128], BF16)
            nc.scalar.dma_start(wv_sb[:], w_v[:])

            for gc in range(SC):
                lo = gc * 512
                hidA = hid_pool.tile([128, HT // 2, 512], BF16, tag="hidA")
                nc.sync.dma_start(hidA[:], hT[:, :HT // 2, lo:lo + 512])
                hidB = hid_pool.tile([128, HT // 2, 512], BF16, tag="hidB")
                nc.scalar.dma_start(hidB[:], hT[:, HT // 2:, lo:lo + 512])

                def hid_slice(ht, js, je):
                    t = hidA if ht < HT // 2 else hidB
                    return t[:, ht % (HT // 2), js:je]

                # v part: [s, vcols] orientation
                for stl in range(4):
                    st = 4 * gc + stl
                    ps_v = psum.tile([128, 512], F32, tag="ps")
                    for ht in range(HT):
                        nc.tensor.matmul(
                            ps_v[:, : HPC * 128],
                            hid_slice(ht, stl * 128, (stl + 1) * 128),
                            wv_sb[:, ht, :],
                            start=(ht == 0), stop=(ht == HT - 1),
                        )
                    nc.vector.tensor_tensor(
                        v_sb[:, st, :], ps_v[:, : HPC * 128], b_v_sb[:],
                        op=mybir.AluOpType.add,
                    )

                # cos/sin slices for this chunk (scaled tables for q)
                csq = cs_pool.tile([128, 2, 512], F32, tag="csq")
                nc.scalar.dma_start(csq[:], cs_q[:, :, lo:lo + 512])
                csk = cs_pool.tile([128, 2, 512], F32, tag="csk")
                nc.scalar.dma_start(csk[:], cs_k[:, :, lo:lo + 512])

                # q/k part: transposed orientation [d, s]
                for ct in range(QK_TILES):
                    n, is_k = ct // 2, ct % 2
                    wt = wqk_pool.tile([128, HT, 128], BF16, tag="wt")
                    (nc.sync if ct % 2 else nc.scalar).dma_start(wt[:], w_qk[ct])
                    cst = csk if is_k else csq
                    dest = kT_sb if is_k else qT_sb
                    ps_qk = psum.tile([128, 512], F32, tag="ps")
                    for ht in range(HT):
                        nc.tensor.matmul(
                            ps_qk[:],
                            wt[:, ht, :],
                            hid_slice(ht, 0, 512),
                            start=(ht == 0), stop=(ht == HT - 1),
                        )
                    qf = rt_pool.tile([128, 512], F32, tag="qf")
                    nc.scalar.activation(
                        qf[:], ps_qk[:], mybir.ActivationFunctionType.Identity,
                        bias=b_qk_sb[:, ct:ct + 1],
                    )
                    qs = rt_pool.tile([128, 512], F32, tag="qs")
                    nc.gpsimd.dma_start(qs[0:32, :], qf[32:64, :])
                    nc.gpsimd.dma_start(qs[32:64, :], qf[0:32, :])
                    nc.gpsimd.dma_start(qs[64:96, :], qf[96:128, :])
                    nc.gpsimd.dma_start(qs[96:128, :], qf[64:96, :])
                    t1 = rt_pool.tile([128, 512], F32, tag="t1")
                    nc.gpsimd.tensor_tensor(
                        t1[:], qs[:], cst[:, 1, :], op=mybir.AluOpType.mult)
                    t2 = rt_pool.tile([128, 512], F32, tag="t2")
                    nc.vector.tensor_tensor(
                        t2[:], qf[:], cst[:, 0, :], op=mybir.AluOpType.mult)
                    nc.vector.tensor_tensor(
                        dest[:, n, lo:lo + 512], t1[:], t2[:],
                        op=mybir.AluOpType.add)

        if debug:
            nc.sync.dma_start(dbg_q[:], qT_sb[:])
            nc.sync.dma_start(dbg_k[:], kT_sb[:])
            nc.sync.dma_start(dbg_v[:], v_sb[:])

        # ---------------- Phase 2+3: attention + dense ----------------
        if phases < 2:
            nc.compile()
            return nc
        with ExitStack() as actx:
            attnw = actx.enter_context(tc.tile_pool(name="attnw", bufs=1))
            wdT_sb = attnw.tile([128, HPC, H], BF16)
            for n in range(HPC):
                nc.scalar.dma_start(wdT_sb[:, n, :], wdT[n])
            ctx_sb = attnw.tile([128, HPC, S], BF16)

            atmp_ctx = ExitStack()
            attn = atmp_ctx.enter_context(tc.tile_pool(name="attn", bufs=1))
            mt_pool = atmp_ctx.enter_context(tc.tile_pool(name="mt", bufs=2))
            ctx_psum = atmp_ctx.enter_context(tc.tile_pool(name="ctxps", bufs=1, space="PSUM"))

            NSEG = 4
            W = S // NSEG
            CPS = W // 512 if W >= 512 else 1  # 512-chunks per segment
            dout = atmp_ctx.enter_context(tc.tile_pool(name="dout", bufs=4))
            dps = atmp_ctx.enter_context(tc.tile_pool(name="dps", bufs=2, space="PSUM"))
            for h2 in range(NSEG):
                for n in range(HPC):
                    sLo, sHi = W * h2, W * (h2 + 1)
                    PTh = attn.tile([128, ST, W], BF16, tag="PT", bufs=3, name="PTh")
                    for t in range(ST):
                        t0 = s0_of(t)
                        if t0 >= sHi:
                            continue
                        s = max(t0, sLo)
                        while s < sHi:
                            w = min(512, sHi - s)
                            ps_s = psum.tile([128, 512], F32, tag="ps", name="ps_s")
                            nc.tensor.matmul(
                                ps_s[:, :w],
                                kT_sb[:, n, 128 * t:128 * (t + 1)],
                                qT_sb[:, n, s:s + w],
                                start=True, stop=True,
                            )
                            nc.scalar.activation(
                                PTh[:, t, s - sLo:s - sLo + w], ps_s[:, :w],
                                mybir.ActivationFunctionType.Exp, scale=coeff)
                            s += w
                        if mode == "causal" and sLo <= t0:
                            if t0 % 512:
                                c0 = t0 - (t0 % 512)
                                nc.vector.memset(PTh[:, t, c0 - sLo:t0 - sLo], 0.0)
                            nc.vector.tensor_tensor(
                                PTh[:, t, t0 - sLo:t0 - sLo + 128],
                                PTh[:, t, t0 - sLo:t0 - sLo + 128], triu_sb[:],
                                op=mybir.AluOpType.mult)
                        elif mode == "general":
                            mt = mt_pool.tile([128, W], BF16, tag="m01")
                            nc.sync.dma_start(mt[:], m01[t][:, sLo:sHi])
                            nc.vector.tensor_tensor(
                                PTh[:, t, :], PTh[:, t, :], mt[:],
                                op=mybir.AluOpType.mult)

                    # row sums for this segment, then 1/l broadcast
                    l_sb = attn.tile([1, W], F32, tag="l", bufs=2, name="l_sb")
                    for cc in range(CPS):
                        c = CPS * h2 + cc
                        tmax = min(ST - 1, 4 * c + 3) if mode == "causal" else ST - 1
                        ps_l = psum.tile([1, 512], F32, tag="ps", name="ps_l")
                        for t in range(tmax + 1):
                            nc.tensor.matmul(
                                ps_l[:], ones_sb[:], PTh[:, t, 512 * cc:512 * (cc + 1)],
                                start=(t == 0), stop=(t == tmax))
                        nc.scalar.copy(l_sb[:, 512 * cc:512 * (cc + 1)], ps_l[:])
                    linv = attn.tile([1, W], F32, tag="linv", bufs=2, name="linv")
                    nc.vector.reciprocal(linv[:], l_sb[:])
                    l_dram = scratch.tile([1, W], F32, tag="ldram")
                    nc.sync.dma_start(l_dram[:], linv[:])
                    linv_b = attn.tile([128, W], F32, tag="linvb", bufs=2, name="linv_b")
                    nc.scalar.dma_start(linv_b[:], l_dram[:].to_broadcast((128, W)))

                    # ctxT[d, s] accumulation over t tiles (this half)
                    ctx_ps = ctx_psum.tile([128, W], F32, tag="ctx", bufs=2, name="ctx_ps")
                    for cc in range(CPS):
                        c = CPS * h2 + cc
                        tmax = min(ST - 1, 4 * c + 3) if mode == "causal" else ST - 1
                        for t in range(tmax + 1):
                            nc.tensor.matmul(
                                ctx_ps[:, 512 * cc:512 * (cc + 1)],
                                v_sb[:, t, 128 * n:128 * (n + 1)],
                                PTh[:, t, 512 * cc:512 * (cc + 1)],
                                start=(t == 0), stop=(t == tmax))
                    nc.vector.tensor_tensor(
                        ctx_sb[:, n, sLo:sHi], ctx_ps[:], linv_b[:],
                        op=mybir.AluOpType.mult)

                # dense for this segment's s tiles (ctx complete across all heads)
                if phases >= 3:
                    for stl in range(W // 128):
                        st = (W * h2) // 128 + stl
                        ot = dout.tile([128, 8, 512], F16, tag="ot")
                        for ch in range(8):
                            ps_o = dps.tile([128, 512], F32, tag="dp", name="ps_o")
                            for nn in range(HPC):
                                nc.tensor.matmul(
                                    ps_o[:],
                                    ctx_sb[:, nn, 128 * st:128 * (st + 1)],
                                    wdT_sb[:, nn, 512 * ch:512 * (ch + 1)],
                                    start=(nn == 0), stop=(nn == HPC - 1))
                            if ch % 2 == 0:
                                nc.scalar.copy(ot[:, ch, :], ps_o[:])
                            else:
                                nc.vector.tensor_copy(ot[:, ch, :], ps_o[:])
                        eng = nc.scalar if st % 2 == 0 else nc.sync
                        eng.dma_start(
                            out_p[128 * st:128 * (st + 1), :],
                            ot[:].rearrange("p c s -> p (c s)"))

            if debug:
                nc.sync.dma_start(dbg_c[:], ctx_sb[:])

            atmp_ctx.close()

    nc.compile()
    return nc


def _prep_inputs(hidden_states, position_ids, attention_mask, layer_id, Wqkv, bqkv, Wd):
    coeff = float(np.asarray(layer_id).item() + 1)
    m = np.asarray(attention_mask).reshape(S, S)
    if not m.any():
        mode = "none"
    elif bool((m == np.triu(np.ones((S, S), bool), 1)).all()):
        mode = "causal"
    else:
        mode = "general"

    hs = np.asarray(hidden_states, np.float32).reshape(S, H)
    hT = np.ascontiguousarray(hs.T.reshape(HT, 128, S).transpose(1, 0, 2)).astype(BF)

    # RoPE tables (match reference fp32 math)
    inv_freq = (1.0 / (10000.0 ** (np.arange(0, ROT, 2, dtype=np.float32) / ROT))).astype(np.float32)
    freqs = np.arange(S, dtype=np.float32)[:, None] * inv_freq[None, :]
    emb = np.concatenate([freqs, freqs], axis=-1)          # [S, 64]
    cos_t, sin_t = np.cos(emb), np.sin(emb)                # fp32 [S, 64]
    pid = np.asarray(position_ids)[0, 0, :].astype(np.int64)
    bid = np.asarray(position_ids)[0, 1, :].astype(np.int64)
    cp, sp = cos_t[pid].T, sin_t[pid].T                    # [64, S]
    cb, sb_ = cos_t[bid].T, sin_t[bid].T
    cos_full = np.concatenate([cp, cb], 0)                 # [128, S]
    # rope via partition-swapped copy: t1[p] = q[p^32] * sin_eff[p]
    sin_eff = np.concatenate([-sp[:32], sp[32:], -sb_[:32], sb_[32:]], 0)
    qscale = np.float32(1.0 / (math.sqrt(HD) * coeff))
    cs_q = np.stack([cos_full * qscale, sin_eff * qscale], 1).astype(np.float32)  # [128,2,S]
    cs_k = np.stack([cos_full, sin_eff], 1).astype(np.float32)

    tri = np.triu(np.ones((128, 128), np.float32)).astype(BF)  # t<=s allowed

    Wq = np.asarray(Wqkv, np.float32).reshape(NH, 3, HD, H)
    bq = np.asarray(bqkv, np.float32).reshape(NH, 3, HD)
    Wd_ = np.asarray(Wd, np.float32)

    per_core = []
    for c in range(NCORES):
        heads = slice(4 * c, 4 * c + 4)
        wqk = Wq[heads, 0:2]                               # [4, 2, 128, H]
        # -> [ct=8, p=128, ht=32, j=128]
        wqk = wqk.reshape(8, HD, HT, 128).transpose(0, 3, 2, 1)   # [8, p, ht, j]
        w_qk = np.ascontiguousarray(wqk).astype(BF)
        wv = Wq[heads, 2]                                  # [4, 128, H]
        wv = wv.reshape(4 * HD, HT, 128).transpose(1, 2, 0)  # [ht, p, 512]
        wv = wv.transpose(1, 0, 2)                         # [p, ht, 512]
        w_v = np.ascontiguousarray(wv).astype(BF)
        b_qk = np.ascontiguousarray(bq[heads, 0:2].reshape(8, 128)).astype(np.float32)
        b_v = np.broadcast_to(bq[heads, 2].reshape(1, 512), (128, 512)).astype(np.float32)
        wd = Wd_[:, 512 * c:512 * (c + 1)]                 # [H, 512]
        wd = np.ascontiguousarray(wd.T.reshape(HPC, 128, H)).astype(BF)
        im = {
            "hT": hT, "w_qk": w_qk, "w_v": w_v, "b_qk": b_qk, "b_v": b_v,
            "cs_q": cs_q, "cs_k": cs_k, "triu": tri, "wdT": wd,
        }
        if mode == "general":
            m01 = (~m).astype(np.float32).T  # [t, s] 1=allowed
            im["m01"] = np.ascontiguousarray(m01.reshape(ST, 128, S)).astype(BF)
        per_core.append(im)
    return coeff, mode, per_core


def kernel(hidden_states, position_ids, attention_mask, layer_id, Wqkv, bqkv, Wd, bd):
    coeff, mode, per_core = _prep_inputs(
        hidden_states, position_ids, attention_mask, layer_id, Wqkv, bqkv, Wd)
    key = (coeff, mode)
    if key not in _PROG_CACHE:
        _PROG_CACHE[key] = _build_program(coeff, mode)
    nc = _PROG_CACHE[key]
    res = run_bass_kernel_spmd(nc, per_core, core_ids=list(range(NCORES)))
    out = np.zeros((S, H), np.float64)
    for r in res.results:
        out += r["out_p"].astype(np.float64)
    out += np.asarray(bd, np.float32).astype(np.float64)
    return out.astype(np.float32).reshape(S, B, H)



# revision 20
# speedup vs baseline: 1.1454x; 1.1454x over previous
"""Trainium2 Bass kernel for ChatGLM attention (S=2048, B=1, H=4096, NH=32, HD=128).

Sharding: tensor-parallel over heads across 8 NeuronCores (4 heads/core).
Each core computes its heads' QKV projection, RoPE, attention, and a
partial dense projection (contraction over its 512 hidden dims); the host
sums the 8 partials and adds the dense bias.

v3: single software-pipelined phase. Per 512-seq chunk gc, the emission
interleaves QKV chains of chunk gc+1 with attention of segment gc and the
dense projection of segment gc-1, so the PE never drains while the
activation engine runs the softmax exponentials.

Key tricks (validated on hw):
- all 12 projection chains (4 heads x q/k/v) produce [d, s]-oriented PSUM;
  RoPE pairs are made partition-XOR-64 by a host-side permutation of the
  q/k weight rows (scores are invariant), so the rotate-half "swap" is two
  cross-partition Activation reads straight out of PSUM (bias fused).
- softmax row-sums use rhs-free-size-1 matmuls (lhsT = P^T tile,
  rhs = ones column), which cost ~0 PE cycles; 1/l is applied as a
  per-partition Activation scale on the [s, d]-oriented context, which is
  then PE-transposed back to [d, s] for the dense matmul.
"""

import math
from contextlib import ExitStack

import ml_dtypes
import numpy as np

import concourse.bacc as bacc
import concourse.mybir as mybir
import concourse.tile as tile
from concourse.bass_utils import run_bass_kernel_spmd

S, B, H, NH, HD = 2048, 1, 4096, 32, 128
ROT = HD // 2
NCORES = 8
HPC = NH // NCORES          # heads per core = 4
NCH = 3 * HPC               # projection chains per core (q/k per head + v) = 12
HT = H // 128               # hidden tiles = 32
ST = S // 128               # seq tiles = 16
SC = S // 512               # 512-wide seq chunks = 4
W = 512                     # attention segment width

F32 = mybir.dt.float32
BF16 = mybir.dt.bfloat16
F16 = mybir.dt.float16
BF = ml_dtypes.bfloat16

_PROG_CACHE = {}

# chain index c: 0..7 -> (head n=c//2, is_k=c%2); 8..11 -> v head c-8
CHAIN_ORDER = [0, 1, 8, 9, 10, 11, 2, 3, 4, 5, 6, 7]


def _build_program(coeff: float, mode: str):
    """mode: 'causal' | 'none' | 'general'"""
    causal = mode == "causal"
    nc = bacc.Bacc("TRN2", target_bir_lowering=False, debug=False)

    hT = nc.dram_tensor("hT", [128, HT, S], BF16, kind="ExternalInput")
    w_all = nc.dram_tensor("w_all", [NCH, 128, HT, 128], BF16, kind="ExternalInput")
    b_all = nc.dram_tensor("b_all", [NCH, 128], F32, kind="ExternalInput")
    cs = nc.dram_tensor("cs", [128, 4, S], BF16, kind="ExternalInput")
    triu = nc.dram_tensor("triu", [128, 128], BF16, kind="ExternalInput")
    eye = nc.dram_tensor("eye", [128, 128], BF16, kind="ExternalInput")
    wdT = nc.dram_tensor("wdT", [HPC, 128, H], BF16, kind="ExternalInput")
    if mode == "general":
        m01 = nc.dram_tensor("m01", [ST, 128, S], BF16, kind="ExternalInput")
    out_p = nc.dram_tensor("out_p", [S, H], F16, kind="ExternalOutput")

    with tile.TileContext(nc) as tc, ExitStack() as ctx:
        const = ctx.enter_context(tc.tile_pool(name="const", bufs=1))
        hidp = ctx.enter_context(tc.tile_pool(name="hid", bufs=1))
        wtp = ctx.enter_context(tc.tile_pool(name="wt", bufs=4))
        csp = ctx.enter_context(tc.tile_pool(name="cs", bufs=1))
        rope = ctx.enter_context(tc.tile_pool(name="rope", bufs=2))
        attn = ctx.enter_context(tc.tile_pool(name="attn", bufs=2))
        ctxp = ctx.enter_context(tc.tile_pool(name="ctxseg", bufs=2))
        dop = ctx.enter_context(tc.tile_pool(name="dout", bufs=4))
        psA = ctx.enter_context(tc.tile_pool(name="psA", bufs=2, space="PSUM"))
        psB = ctx.enter_context(tc.tile_pool(name="psB", bufs=2, space="PSUM"))
        psS = ctx.enter_context(tc.tile_pool(name="psS", bufs=2, space="PSUM"))
        if mode == "general":
            mtp = ctx.enter_context(tc.tile_pool(name="mt", bufs=2))

        b_sb = const.tile([128, NCH], F32)
        triu_sb = const.tile([128, 128], BF16)
        eye_sb = const.tile([128, 128], BF16)
        ones_sb = const.tile([128, 1], BF16)
        nc.vector.memset(ones_sb[:], 1.0)
        qT_sb = const.tile([128, HPC, S], BF16)
        kT_sb = const.tile([128, HPC, S], BF16)
        v_sb = const.tile([128, ST, HPC * 128], BF16)
        wdT_sb = const.tile([128, HPC, H], BF16)

        cur_hid = {}
        cur_cs = [None]
        ctx_segs = {}

        def load_hid(gc):
            lo = 512 * gc
            for q in range(8):
                t = hidp.tile([128, 4, 512], BF16, tag=f"hid{q}", name=f"hid{q}")
                eng = nc.sync if q % 2 == 0 else nc.gpsimd
                eng.dma_start(t[:], hT[:, 4 * q:4 * q + 4, lo:lo + 512])
                cur_hid[q] = t

        def cur_hid_dma(gc, q):
            t = hidp.tile([128, 4, 512], BF16, tag=f"hid{q}", name=f"hid{q}")
            cur_hid[q] = t
            return t[:]

        def load_cs(gc):
            lo = 512 * gc
            t = csp.tile([128, 4, 512], BF16, tag="cs", name="cs_t")
            nc.gpsimd.dma_start(t[:], cs[:, :, lo:lo + 512])
            cur_cs[0] = t

        def chain_mms(gc, c, ps, wt, q_lo=0, q_hi=4):
            for q in range(q_lo, q_hi):
                for hl in range(8):
                    ht = 8 * q + hl
                    nc.tensor.matmul(
                        ps[:], wt[:, ht, :], cur_hid[ht // 4][:, ht % 4, :],
                        start=(ht == 0), stop=(ht == HT - 1))

        def rope_tail(gc, c, ps):
            n, is_k = c // 2, c % 2
            lo = 512 * gc
            cst = cur_cs[0]
            qf = rope.tile([128, 512], BF16, tag="qf", name="qf")
            nc.scalar.activation(qf[:], ps[:],
                                 mybir.ActivationFunctionType.Identity,
                                 bias=b_sb[:, c:c + 1])
            qs = rope.tile([128, 512], BF16, tag="qs", name="qs")
            nc.scalar.activation(qs[0:64, :], ps[64:128, :],
                                 mybir.ActivationFunctionType.Identity,
                                 bias=b_sb[64:128, c:c + 1])
            nc.scalar.activation(qs[64:128, :], ps[0:64, :],
                                 mybir.ActivationFunctionType.Identity,
                                 bias=b_sb[0:64, c:c + 1])
            t2 = rope.tile([128, 512], BF16, tag="t2", bufs=1, name="t2")
            nc.vector.tensor_tensor(t2[:], qf[:], cst[:, 2 * is_k, :],
                                    op=mybir.AluOpType.mult)
            t1 = rope.tile([128, 512], BF16, tag="t1", bufs=1, name="t1")
            nc.vector.tensor_tensor(t1[:], qs[:], cst[:, 2 * is_k + 1, :],
                                    op=mybir.AluOpType.mult)
            dest = kT_sb if is_k else qT_sb
            nc.vector.tensor_tensor(dest[:, n, lo:lo + 512], t1[:], t2[:],
                                    op=mybir.AluOpType.add)

        def v_tail(gc, c, ps):
            nv = c - 8
            vst = rope.tile([128, 512], BF16, tag="vst", name="vst")
            nc.scalar.activation(vst[:], ps[:],
                                 mybir.ActivationFunctionType.Identity,
                                 bias=b_sb[:, c:c + 1])
            for j in range(4):
                vtr = psS.tile([128, 128], BF16, tag="tr", name="vtr")
                nc.tensor.transpose(vtr[:], vst[:, 128 * j:128 * (j + 1)], eye_sb[:])
                if j % 2:
                    nc.vector.tensor_copy(
                        v_sb[:, 4 * gc + j, 128 * nv:128 * (nv + 1)], vtr[:])
                else:
                    nc.scalar.copy(
                        v_sb[:, 4 * gc + j, 128 * nv:128 * (nv + 1)], vtr[:])

        def gen_chain(gc, c):
            wt = wtp.tile([128, HT, 128], BF16, tag="wt", name="wt")
            nc.scalar.dma_start(wt[:], w_all[c])
            ps = psA.tile([128, 512], F32, tag="ps", name="ps_c")
            for q in range(4):
                chain_mms(gc, c, ps, wt, q, q + 1)
                yield
            if c < 8:
                rope_tail(gc, c, ps)
            else:
                v_tail(gc, c, ps)
            yield

        def gen_chains(gc, cs_list):
            for c in cs_list:
                yield from gen_chain(gc, c)

        def t_top(h2):
            return 4 * h2 + 4 if causal else ST

        def fin(ctxs, n, ctxn, u, h2):
            trp = psS.tile([128, 128], BF16, tag="tr", name="trp")
            nc.tensor.transpose(trp[:], ctxn[:], eye_sb[:])
            if u % 2:
                nc.vector.tensor_copy(ctxs[:, n, 128 * u:128 * (u + 1)], trp[:])
            else:
                nc.scalar.copy(ctxs[:, n, 128 * u:128 * (u + 1)], trp[:])

        def gen_att(h2):
            sLo, sHi = W * h2, W * (h2 + 1)
            ctxs = ctx_segs[h2]
            for n in range(HPC):
                PT = attn.tile([128, ST, W], BF16, tag="PT", name="PT")
                for t in range(t_top(h2)):
                    t0 = 128 * t
                    s = max(t0, sLo) if causal else sLo
                    while s < sHi:
                        w = min(512, sHi - s)
                        pool = psA if (h2 == SC - 1 and t % 2 == 1) else psB
                        ps = pool.tile([128, 512], F32, tag="ps", name="ps_s")
                        nc.tensor.matmul(
                            ps[:, :w],
                            kT_sb[:, n, 128 * t:128 * (t + 1)],
                            qT_sb[:, n, s:s + w],
                            start=True, stop=True)
                        nc.scalar.activation(
                            PT[:, t, s - sLo:s - sLo + w], ps[:, :w],
                            mybir.ActivationFunctionType.Exp, scale=coeff)
                        s += w
                    if causal and t0 >= sLo:
                        nc.vector.tensor_tensor(
                            PT[:, t, t0 - sLo:t0 - sLo + 128],
                            PT[:, t, t0 - sLo:t0 - sLo + 128], triu_sb[:],
                            op=mybir.AluOpType.mult)
                    elif mode == "general":
                        mt = mtp.tile([128, W], BF16, tag="m01", name="mt")
                        nc.sync.dma_start(mt[:], m01[t][:, sLo:sHi])
                        nc.vector.tensor_tensor(
                            PT[:, t, :], PT[:, t, :], mt[:],
                            op=mybir.AluOpType.mult)
                    yield
                # row sums via free-size-1 matmuls, then 1/l
                l_sm = psS.tile([128, 132], F32, tag="sm", name="l_sm")
                for u in range(4):
                    tmax = 4 * h2 + u if causal else ST - 1
                    for t in range(tmax + 1):
                        nc.tensor.matmul(
                            l_sm[:, 128 + u:129 + u],
                            PT[:, t, 128 * u:128 * (u + 1)], ones_sb[:],
                            start=(t == 0), stop=(t == tmax))
                linv = attn.tile([128, 4], F32, tag="linv", name="linv")
                nc.vector.reciprocal(linv[:], l_sm[:, 128:132])
                yield
                fins = []
                for u in range(4):
                    tmax = 4 * h2 + u if causal else ST - 1
                    ctx_sm = psS.tile([128, 132], F32, tag="sm", name="ctx_sm")
                    ctx_ps = ctx_sm[:, 0:128]
                    for t in range(tmax + 1):
                        nc.tensor.matmul(
                            ctx_ps[:],
                            PT[:, t, 128 * u:128 * (u + 1)],
                            v_sb[:, t, 128 * n:128 * (n + 1)],
                            start=(t == 0), stop=(t == tmax))
                    ctxn = attn.tile([128, 128], BF16, tag="ctxn", bufs=4, name="ctxn")
                    nc.vector.tensor_scalar(ctxn[:], ctx_ps[:], linv[:, u:u + 1],
                                            None, op0=mybir.AluOpType.mult)
                    if len(fins) > 1:
                        fin(*fins.pop(0))
                    fins.append((ctxs, n, ctxn, u, h2))
                    yield
                while fins:
                    fin(*fins.pop(0))
                    yield

        def gen_dense(h2, st_list, last=False):
            ctxs = ctx_segs[h2]
            for stl in st_list:
                st = 4 * h2 + stl
                for ch in range(8):
                    ps_o = psB.tile([128, 512], F32, tag="ps", name="ps_o")
                    for nn in range(HPC):
                        nc.tensor.matmul(
                            ps_o[:],
                            ctxs[:, nn, 128 * stl:128 * (stl + 1)],
                            wdT_sb[:, nn, 512 * ch:512 * (ch + 1)],
                            start=(nn == 0), stop=(nn == HPC - 1))
                    ot = dop.tile([128, 512], F16, tag="ot", name="ot")
                    if ch % 2 == 0:
                        nc.scalar.copy(ot[:], ps_o[:])
                    else:
                        nc.vector.tensor_copy(ot[:], ps_o[:])
                    if last:
                        eng = nc.scalar if ch % 2 else nc.sync
                    else:
                        eng = nc.gpsimd if ch % 2 else nc.sync
                    eng.dma_start(
                        out_p[128 * st:128 * (st + 1), 512 * ch:512 * (ch + 1)],
                        ot[:])
                    yield

        def interleave(streams, weights):
            alive = [(g, w) for g, w in zip(streams, weights)]
            while alive:
                nxt = []
                for g, w in alive:
                    ok = True
                    for _ in range(w):
                        try:
                            next(g)
                        except StopIteration:
                            ok = False
                            break
                    if ok:
                        nxt.append((g, w))
                alive = nxt

        # ---------------- emission ----------------
        # QKV(0): first four chains interleaved at hid-eighth granularity, with
        # weight tiles quarter-split too, so the PE starts ~3us in and is never
        # DMA-starved. DMA order matters: the sim serializes transfers, so
        # first-needed bytes go first.
        NSP = 4
        wts = []
        for ci in range(NSP):
            wts.append(wtp.tile([128, HT, 128], BF16, tag="wt", name="wt"))
        sps = []
        for ci in range(NSP):
            pool = psA if ci < 2 else psB
            sps.append(pool.tile([128, 512], F32, tag="ps", name="ps_c"))
        def wtq_dma(ci, qq):
            nc.scalar.dma_start(
                wts[ci][:, 8 * qq:8 * qq + 8, :],
                w_all[CHAIN_ORDER[ci], :, 8 * qq:8 * qq + 8, :])

        def hid_dma(q):
            eng = nc.sync if q % 2 == 0 else nc.gpsimd
            eng.dma_start(cur_hid_dma(0, q), hT[:, 4 * q:4 * q + 4, 0:512])

        wtq_dma(0, 0)
        hid_dma(0)
        wtq_dma(1, 0)
        nc.gpsimd.dma_start(eye_sb[:], eye[:])
        hid_dma(1)
        wtq_dma(2, 0)
        wtq_dma(3, 0)
        nc.gpsimd.dma_start(b_sb[:], b_all[:].rearrange("t p -> p t"))
        for qq in range(1, 4):
            hid_dma(2 * qq)
            wtq_dma(0, qq)
            hid_dma(2 * qq + 1)
            wtq_dma(1, qq)
            wtq_dma(2, qq)
            wtq_dma(3, qq)
        load_cs(0)
        nc.gpsimd.dma_start(triu_sb[:], triu[:])
        for q in range(4):
            for ci in range(NSP):
                chain_mms(0, CHAIN_ORDER[ci], sps[ci], wts[ci], q, q + 1)
        for ci in range(NSP):
            c = CHAIN_ORDER[ci]
            if c < 8:
                rope_tail(0, c, sps[ci])
            else:
                v_tail(0, c, sps[ci])
        for _ in gen_chains(0, CHAIN_ORDER[NSP:]):
            pass
        for nn in range(HPC):
            nc.gpsimd.dma_start(wdT_sb[:, nn, :], wdT[nn])

        for gc in range(1, 5):
            h2 = gc - 1
            ctx_segs[h2] = ctxp.tile([128, HPC, W], BF16, tag="ctxs", name="ctxs")
            streams, weights = [], []
            if gc <= 3:
                load_cs(gc)
                load_hid(gc)
                chains = list(CHAIN_ORDER)
                if gc == 3:
                    chains, tail_chains = chains[:-3], chains[-3:]
                streams.append(gen_chains(gc, chains))
                weights.append(1)
            else:
                streams.append(gen_chains(3, tail_chains))
                weights.append(1)
            streams.append(gen_att(h2))
            weights.append(2 if gc <= 3 else 3)
            if h2 >= 1:
                streams.append(gen_dense(h2 - 1, range(4)))
                weights.append(1)
            interleave(streams, weights)

        # dense for the last segment
        for _ in gen_dense(3, range(4), last=True):
            pass

    nc.compile()
    return nc


def _prep_inputs(hidden_states, position_ids, attention_mask, layer_id, Wqkv, bqkv, Wd):
    coeff = float(np.asarray(layer_id).item() + 1)
    m = np.asarray(attention_mask).reshape(S, S)
    if not m.any():
        mode = "none"
    elif bool((m == np.triu(np.ones((S, S), bool), 1)).all()):
        mode = "causal"
    else:
        mode = "general"

    hs = np.asarray(hidden_states, np.float32).reshape(S, H)
    hT = np.ascontiguousarray(hs.T.reshape(HT, 128, S).transpose(1, 0, 2)).astype(BF)

    # RoPE tables (fp32, matching the reference) with the XOR-64 partner
    # permutation: new row order [q1_lo, q2_lo, q1_hi, q2_hi].
    OLDIDX = np.concatenate([np.arange(0, 32), np.arange(64, 96),
                             np.arange(32, 64), np.arange(96, 128)])
    inv_freq = (1.0 / (10000.0 ** (np.arange(0, ROT, 2, dtype=np.float32) / ROT))).astype(np.float32)
    freqs = np.arange(S, dtype=np.float32)[:, None] * inv_freq[None, :]
    emb = np.concatenate([freqs, freqs], axis=-1)          # [S, 64]
    cos_t, sin_t = np.cos(emb), np.sin(emb)                # fp32 [S, 64]
    pid = np.asarray(position_ids)[0, 0, :].astype(np.int64)
    bid = np.asarray(position_ids)[0, 1, :].astype(np.int64)
    cp, sp = cos_t[pid].T, sin_t[pid].T                    # [64, S]
    cb, sb_ = cos_t[bid].T, sin_t[bid].T
    cos_new = np.concatenate([cp[0:32], cb[0:32], cp[32:64], cb[32:64]], 0)
    sin_eff = np.concatenate([-sp[0:32], -sb_[0:32], sp[32:64], sb_[32:64]], 0)
    qscale = np.float32(1.0 / (math.sqrt(HD) * coeff))
    cs = np.stack([cos_new * qscale, sin_eff * qscale, cos_new, sin_eff], 1)
    cs = np.ascontiguousarray(cs).astype(BF)               # [128, 4, S]

    tri = np.triu(np.ones((128, 128), np.float32)).astype(BF)
    eye = np.eye(128, dtype=np.float32).astype(BF)

    Wq = np.asarray(Wqkv, np.float32).reshape(NH, 3, HD, H)
    bq = np.asarray(bqkv, np.float32).reshape(NH, 3, HD)
    Wd_ = np.asarray(Wd, np.float32)

    per_core = []
    for c in range(NCORES):
        w_all = np.empty((NCH, 128, HT, 128), np.float32)
        b_all = np.empty((NCH, 128), np.float32)
        for n in range(HPC):
            head = 4 * c + n
            for is_k in range(2):
                wm = Wq[head, is_k][OLDIDX]                # [128 d(perm), H]
                w_all[2 * n + is_k] = wm.reshape(HD, HT, 128).transpose(2, 1, 0)
                b_all[2 * n + is_k] = bq[head, is_k][OLDIDX]
            wv = Wq[head, 2]                               # [128 d, H]
            w_all[8 + n] = wv.reshape(HD, HT, 128).transpose(2, 1, 0)
            b_all[8 + n] = bq[head, 2]
        wd = Wd_[:, 512 * c:512 * (c + 1)]                 # [H, 512]
        wd = np.ascontiguousarray(wd.T.reshape(HPC, 128, H)).astype(BF)
        im = {
            "hT": hT, "w_all": np.ascontiguousarray(w_all).astype(BF),
            "b_all": b_all, "cs": cs, "triu": tri, "eye": eye, "wdT": wd,
        }
        if mode == "general":
            m01 = (~m).astype(np.float32).T  # [t, s] 1=allowed
            im["m01"] = np.ascontiguousarray(m01.reshape(ST, 128, S)).astype(BF)
        per_core.append(im)
    return coeff, mode, per_core


def kernel(hidden_states, position_ids, attention_mask, layer_id, Wqkv, bqkv, Wd, bd):
    coeff, mode, per_core = _prep_inputs(
        hidden_states, position_ids, attention_mask, layer_id, Wqkv, bqkv, Wd)
    key = (coeff, mode)
    if key not in _PROG_CACHE:
        _PROG_CACHE[key] = _build_program(coeff, mode)
    nc = _PROG_CACHE[key]
    res = run_bass_kernel_spmd(nc, per_core, core_ids=list(range(NCORES)))
    out = np.zeros((S, H), np.float64)
    for r in res.results:
        out += r["out_p"].astype(np.float64)
    out += np.asarray(bd, np.float32).astype(np.float64)
    return out.astype(np.float32).reshape(S, B, H)
